# revision 53
# baseline (speedup 1.0000x reference)
"""Trainium2 Bass kernel for nn_O3TensorProductWeighted.

Computes, for each sample e:
    h  = relu(relu(weight @ W0 + b0) @ W1 + b1)           # [64]
    w  = h @ W2 + b2                                      # [36864] (never materialized)
    out0 = PW0*(einsum(Wa,s1)*s2 + I3*einsum(Wd,dot12))
    out1 = PW1*I3*(einsum(Wb,s1) x v2 + einsum(Wc,v1)*s2)
    out  = concat(out0, out1)/SQRT_K ; out[:128] += bias

Strategy: reassociate each einsum against the (k,u)-joint contraction of the
per-sample Khatri-Rao product h (x) x, so everything becomes dense matmuls
over shared W2 chunk weights. Samples ride on PSUM partitions: each matmul
takes a [128 contraction, 128 samples] KR slice as the stationary and
streams W2 chunk columns as the moving operand, so PE cost equals true
output columns (bf16 roofline) and the result lands directly in [E, 320]
layout - no final transposes. The D path (dot12 x h against Wd, ~36% of the
output norm) runs as fp8-e4m3 DoubleRow matmuls covering a chunk pair per
instruction at the doubled fp8 PE rate, adding ~1.1e-2 rel err against the
2e-2 gate. The per-chunk h-broadcast is a partition-replicating SBUF->SBUF
DMA from h2 (for the first KPE chunks, a PE matmul against replicated W1
columns straight from h1, skipping the h2 dependency); the KR multiply is
split DVE (blocks 0-3, bf16 2x) / Pool (block 4 bf16 + block 5 fp8), with
blocks 2-3 shifting to Pool for the final chunks so both engines drain
together. b2/bias opener matmuls run mid-loop (accumulation is commutative),
keeping their params off the startup critical path and the openers off the
tail; the output bias row rides a ones contraction row in the D opener.
Epilogue: s2/v2 are per-partition scalars -> ACT scale + fused DVE adds per
128-sample block, interleaved with the last chunk's matmuls. Weight slabs
stream on the SP ring in consumption order; fin0 loads via Pool SWDGE; the
ACT ring carries the broadcast DMAs. Pure data parallel over 8 cores, 512
samples each.
"""

import dataclasses
import sys

sys.path.insert(0, "/opt/trn_rl_repo")

from contextlib import ExitStack

import ml_dtypes
import numpy as np

import concourse.bacc as bacc
import concourse.bass as bass
import concourse.tile as tile
from concourse import mybir
from concourse.bass_utils import run_bass_kernel_spmd

BF16 = mybir.dt.bfloat16
F32 = mybir.dt.float32
F8 = mybir.dt.float8e4
BF16_NP = ml_dtypes.bfloat16
F8_NP = ml_dtypes.float8_e4m3fn

N_CORES = 8
N = 4096
E = N // N_CORES  # 512 samples per core
EB = E // 128     # 4 sample blocks of 128 (PSUM partition dim)

MUL0, MUL1 = 128, 64
N1 = MUL0 * MUL0          # 16384
N2 = MUL0 * MUL1          # 8192
N3 = MUL1 * MUL1          # 4096
I3 = float(1.0 / np.sqrt(3.0))
# PW0/SQRT_K == 1.0 and PW1*I3/SQRT_K == 1.0 exactly; only I3 remains on D,
# folded into the wdd/bdb host tensors.

G = 32   # chunks; chunk g covers k in {2g, 2g+1} x 64 u-values (128 rows)
KPE = 5  # chunks whose h-broadcast runs on PE while the DMA path warms up


def _build_nc():
    nc = bacc.Bacc(None)

    def dram(name, shape, dt=BF16):
        return nc.declare_dram_parameter(name, shape, dt, isOutput=False)

    # per-core inputs
    # fused KR x-operand: [s1lo2 | s1hi2 | vs0 | vs1 | vs2 | d22], each [128, E]
    fin0_d = dram("fin0", [128, 6 * E])
    pk16_d = dram("pk16", [16, 64 + E])        # [w0 | wT]
    pkw_d = dram("pkw", [64, 64 + 128 * KPE])  # [w1 | wg1K]
    pk65_d = dram("pk65", [65, E + 192])       # [d2o | bdb | bc2(64 rows)]
    pk128_d = dram("pk128", [128, E + 320])    # [s1t | bab | ident]
    sv_d = dram("sv", [128, 6 + KPE], F32)  # [s2 per block | bg1K | b0 | b1]
    v2b3_d = dram("v2b3", [128, EB * 192], F32)  # v2_i bcast per sample block

    # shared W2 chunk tensors; the D path runs in fp8 DoubleRow (its term is
    # ~36% of the output norm, adding ~1.1e-2 rel err - inside the 2e-2 gate)
    wab_lo_d = dram("wab_lo", [128, G * 192])
    wab_hi_d = dram("wab_hi", [128, G * 192])
    wcc_d = dram("wcc", [128, G * 64])
    wc2_d = dram("wc2", [128, G * 64], F8)
    wdd_d = dram("wdd", [128, G * 128], F8)

    outp_d = nc.declare_dram_parameter("outp", [E, 320], F32, isOutput=True)

    with tile.TileContext(nc) as tc, ExitStack() as ctx:
        const = ctx.enter_context(tc.tile_pool(name="const", bufs=1))
        work = ctx.enter_context(tc.tile_pool(name="work", bufs=1))
        bct_pool = ctx.enter_context(tc.tile_pool(name="bct", bufs=KPE + 2))
        pt_pool = ctx.enter_context(tc.tile_pool(name="pt", bufs=KPE + 2))
        pt5_pool = ctx.enter_context(tc.tile_pool(name="pt5", bufs=4))
        out_pool = ctx.enter_context(tc.tile_pool(name="outs", bufs=4))
        ps_acc = ctx.enter_context(tc.tile_pool(name="ps_acc", bufs=1, space="PSUM"))
        ps_mlp = ctx.enter_context(tc.tile_pool(name="ps_mlp", bufs=1, space="PSUM"))
        ps_bc = ctx.enter_context(tc.tile_pool(name="ps_bc", bufs=2, space="PSUM"))

        def tileof(dparam):
            return const.tile(dparam.shape, dparam.dtype, name=f"t_{dparam.name}")

        # sv (scales + all MLP biases) rides the idle ACT ring at t=0 so the
        # startup chain never waits on the SP ring behind pk16/pkw.
        sv_t = tileof(sv_d)
        nc.scalar.dma_start(sv_t[:], sv_d[:])
        # SP ring: MLP/broadcast params first, then W2 slabs in consumption
        # order (fine slabs first so chunk 0 weights land just in time).
        pk16_t = tileof(pk16_d)
        nc.sync.dma_start(pk16_t[:, 0:320], pk16_d[:, 0:320])
        nc.sync.dma_start(pk16_t[:, 320:64 + E], pk16_d[:, 320:64 + E])
        pkw_t = tileof(pkw_d)
        nc.sync.dma_start(pkw_t[:], pkw_d[:])
        wab_hi_t = tileof(wab_hi_d)
        wab_lo_t = tileof(wab_lo_d)
        wcc_t = tileof(wcc_d)
        wc2_t = tileof(wc2_d)
        wdd_t = tileof(wdd_d)
        slabs = [(0, 4), (4, 8), (8, 16), (16, 24), (24, 32)]
        for si, (g0, g1) in enumerate(slabs):
            for t, dp, w in ((wab_hi_t, wab_hi_d, 192), (wab_lo_t, wab_lo_d, 192),
                             (wcc_t, wcc_d, 64), (wc2_t, wc2_d, 64),
                             (wdd_t, wdd_d, 128)):
                nc.sync.dma_start(t[:, g0 * w:g1 * w], dp[:, g0 * w:g1 * w])
            if si == 1:
                pk128_t = tileof(pk128_d)
                nc.sync.dma_start(pk128_t[:], pk128_d[:])
            elif si == 2:
                pk65_t = tileof(pk65_d)
                nc.sync.dma_start(pk65_t[:], pk65_d[:])
            elif si == 3:
                v2b3_t = tileof(v2b3_d)
                nc.sync.dma_start(v2b3_t[:], v2b3_d[:])

        # Pool self-loads fin0 via SWDGE before its multiply work begins
        fin0_t = tileof(fin0_d)
        nc.gpsimd.dma_start(fin0_t[:, 4 * E:6 * E], fin0_d[:, 4 * E:6 * E])
        nc.gpsimd.dma_start(fin0_t[:, 0:4 * E], fin0_d[:, 0:4 * E])

        # ACT warmup: pay the activation-table load before the MLP needs ACT
        warm = work.tile([1, 8], F32, name="warm")
        nc.vector.memset(warm[:], 0.0)
        nc.scalar.activation(warm[:], warm[:],
                             mybir.ActivationFunctionType.Relu, bias=0.0,
                             scale=1.0)

        # MLP layer 1: h1 = relu(W0.T @ wT + b0) : [64, E], E-split (with
        # separate PSUM tiles, avoiding a false WAR between the halves) so
        # the chunk-0 chain can proceed on the first half early
        h1_t = work.tile([64, E], BF16, name="h1")
        ps_h1 = ps_mlp.tile([128, 256], F32, name="ps_h1")
        for hi, (c0, c1) in enumerate(((0, 256), (256, E))):
            half = ps_h1[64 * hi:64 * (hi + 1), :]
            nc.tensor.matmul(half, pk16_t[:, 0:64],
                             pk16_t[:, 64 + c0:64 + c1], start=True, stop=True,
                             skip_group_check=True)
            nc.scalar.activation(h1_t[:, c0:c1], half,
                                 mybir.ActivationFunctionType.Relu,
                                 bias=sv_t[0:64, 4 + KPE:5 + KPE], scale=1.0)

        # persistent PSUM accumulators: one bank per 128-sample block,
        # columns [A(128) | B(64) | C0 C1 C2 (64 each) | D(128)]
        acc = [ps_acc.tile([128, 512], F32, name=f"acc{eb}") for eb in range(EB)]

        h2_t = work.tile([64, E], BF16, name="h2")

        # per-chunk KR product. First KPE chunks fuse layer 2 into a PE
        # replicated-W1-column matmul on h1 (no h2 dependency); later chunks
        # use a partition-replicating DMA from h2 on the ACT ring. Blocks 0-3
        # multiply on DVE (bf16, 2x), block 4 on Pool (bf16), block 5 on Pool
        # into the fp8 chunk-pair tile feeding the DoubleRow D matmuls.
        pt45_cur = [None]

        def emit_pt(g):
            bct = bct_pool.tile([128, E], BF16, tag="bct")
            if g < KPE:
                psb = ps_bc.tile([128, E], F32, tag="psb")
                halves = ((0, 256), (256, E)) if g == 0 else ((0, E),)
                for c0, c1 in halves:
                    nc.tensor.matmul(psb[:, c0:c1],
                                     pkw_t[:, 64 + 128 * g:64 + 128 * (g + 1)],
                                     h1_t[:, c0:c1], start=c0 == 0,
                                     stop=c1 == E, skip_group_check=True)
                    if g == 0 and c0 == 0:
                        # DVE computes the first quarter's relu itself so its
                        # stream starts without waiting on the ACT queue
                        nc.vector.tensor_scalar(
                            bct[:, 0:256], psb[:, 0:256],
                            sv_t[:, 4 + g:5 + g], 0.0,
                            mybir.AluOpType.add, mybir.AluOpType.max)
                    else:
                        nc.scalar.activation(bct[:, c0:c1], psb[:, c0:c1],
                                             mybir.ActivationFunctionType.Relu,
                                             bias=sv_t[:, 4 + g:5 + g],
                                             scale=1.0)
            else:
                src = h2_t[2 * g:2 * g + 2, :]
                src_b = dataclasses.replace(src, ap=[src.ap[0], [0, 64], [1, E]])
                nc.scalar.dma_start(bct[:], src_b)

            pt = pt_pool.tile([128, 4 * E], BF16, tag="pt")
            # chunk 0's DVE multiply is split so the AB matmuls (blocks 0/1)
            # can start while blocks 2/3 are still being produced. Pool runs
            # ~200ns/chunk ahead of DVE, so for the last 4 chunks blocks 2-3
            # shift to Pool and both engines finish the KR stream early.
            if g == 0:
                dve_parts, pool_parts = (), ()
                for c0, c1 in ((0, 256), (256, E)):
                    for b0_, b1_ in ((0, 2), (2, 4)):
                        nb = b1_ - b0_
                        bctb = dataclasses.replace(
                            bct[:, c0:c1],
                            ap=[bct[:, c0:c1].ap[0], [0, nb], [1, c1 - c0]])
                        nc.vector.tensor_mul(
                            pt[:, b0_ * E:b1_ * E].rearrange(
                                "p (b e) -> p b e", b=nb)[:, :, c0:c1],
                            fin0_t[:, b0_ * E:b1_ * E].rearrange(
                                "p (b e) -> p b e", b=nb)[:, :, c0:c1], bctb)
            elif g >= G - 6:
                dve_parts, pool_parts = ((0, 2),), ((2, 4),)
            else:
                dve_parts, pool_parts = ((0, 4),), ()
            for b0_, b1_ in dve_parts:
                nb = b1_ - b0_
                bctb = dataclasses.replace(
                    bct[:], ap=[bct[:].ap[0], [0, nb], [1, E]])
                nc.vector.tensor_mul(
                    pt[:, b0_ * E:b1_ * E].rearrange("p (b e) -> p b e", b=nb),
                    fin0_t[:, b0_ * E:b1_ * E].rearrange(
                        "p (b e) -> p b e", b=nb), bctb)
            for b0_, b1_ in pool_parts:
                if b0_ == 4:
                    continue
                nb = b1_ - b0_
                bctb = dataclasses.replace(
                    bct[:], ap=[bct[:].ap[0], [0, nb], [1, E]])
                nc.gpsimd.tensor_mul(
                    pt[:, b0_ * E:b1_ * E].rearrange("p (b e) -> p b e", b=nb),
                    fin0_t[:, b0_ * E:b1_ * E].rearrange(
                        "p (b e) -> p b e", b=nb), bctb)
            if g % 2 == 0:
                pt45_cur[0] = pt5_pool.tile([128, 4 * E], F8, tag="pt45",
                                            name="pt45")
            ptf = pt45_cur[0]
            dst = dataclasses.replace(
                ptf[:, (g % 2) * E:(g % 2 + 1) * E],
                ap=[ptf[:].ap[0], [2 * E, 2], [1, E]])
            bct2 = dataclasses.replace(
                bct[:], ap=[bct[:].ap[0], [0, 2], [1, E]])
            nc.gpsimd.tensor_mul(
                dst, fin0_t[:, 4 * E:6 * E].rearrange("p (b e) -> p b e", b=2),
                bct2)
            return pt, ptf[:, 0:2 * E], ptf[:, 2 * E:4 * E]

        # prefill so PE never waits on the broadcast/multiply chain mid-loop;
        # layer 2 (h2, feeding the DMA-replicated chunks) is slotted between
        # the first prefill chunks so it stays off the critical path.
        pre = [emit_pt(0), emit_pt(1)]
        ps_h2 = ps_mlp.tile([64, E], F32, name="ps_h2")
        nc.tensor.matmul(ps_h2[:], pkw_t[:, 0:64], h1_t[:], start=True, stop=True)
        nc.scalar.activation(h2_t[:], ps_h2[:], mybir.ActivationFunctionType.Relu,
                             bias=sv_t[0:64, 5 + KPE:6 + KPE], scale=1.0)
        pre += [emit_pt(g) for g in range(2, KPE)]

        # per sample block finisher: openers (b2 bias terms + output bias via
        # the ones row; accumulation is commutative so they close each group),
        # then the ACT/DVE epilogue:
        # out0 = A*s2 + (I3*D + bias) ; out1_i = B*v2_i + C_i
        mult_op = mybir.AluOpType.mult
        add_op = mybir.AluOpType.add

        tAs, tBs = [], []

        def finish_eb(eb):
            # phase 1: per-partition scale factors into bf16 staging tiles
            tA = work.tile([128, 128], BF16, tag=f"tA{eb}", name=f"tA{eb}")
            nc.scalar.mul(tA[:], acc[eb][:, 0:128], sv_t[:, eb:eb + 1])
            tB = work.tile([128, 192], BF16, tag=f"tB{eb}", name=f"tB{eb}")
            accB = acc[eb][:, 128:192]
            accB3 = dataclasses.replace(accB, ap=[accB.ap[0], [0, 3], [1, 64]])
            nc.vector.tensor_mul(
                tB[:].rearrange("p (i w) -> p i w", i=3),
                v2b3_t[:, eb * 192:(eb + 1) * 192].rearrange(
                    "p (i w) -> p i w", i=3), accB3)
            tAs.append(tA); tBs.append(tB)

        def finish2_eb(eb):
            # phase 2: identity matmuls add the scaled terms into the PSUM
            # accumulators (PE is cheap: cost = output columns), then plain
            # ACT/DVE evacuations feed the output DMA
            sl = bass.ts(eb, 128)
            ident = pk128_t[:, E + 192:E + 320]
            nc.tensor.matmul(acc[eb][:, 384:512], ident, tAs[eb][:],
                             start=False, stop=False, skip_group_check=True)
            nc.tensor.matmul(acc[eb][:, 192:384], ident, tBs[eb][:],
                             start=False, stop=False, skip_group_check=True)
            outS = out_pool.tile([128, 320], F32, tag="outS")
            nc.scalar.copy(outS[:, 0:128], acc[eb][:, 384:512])
            o1v = outS[:, 128:320].rearrange("p (w i) -> p i w", i=3)
            nc.vector.tensor_copy(
                o1v, acc[eb][:, 192:384].rearrange("p (i w) -> p i w", i=3))
            (nc.sync if eb % 2 == 0 else nc.scalar).dma_start(
                outp_d[sl, :], outS[:])

        def emit_openers():
            # b2 bias terms + output bias (ones row): accumulation is
            # commutative, so these run mid-loop once their params are loaded
            for eb in range(EB):
                sl = bass.ts(eb, 128)
                nc.tensor.matmul(acc[eb][:, 0:192], pk128_t[:, sl],
                                 pk128_t[:, E:E + 192], start=False,
                                 stop=False, skip_group_check=True)
                for i in range(3):
                    nc.tensor.matmul(acc[eb][:, 192 + 64 * i:256 + 64 * i],
                                     fin0_t[0:64, (2 + i) * E + eb * 128:
                                            (2 + i) * E + eb * 128 + 128],
                                     pk65_t[0:64, E + 128:E + 192],
                                     start=False, stop=False,
                                     skip_group_check=True)
                nc.tensor.matmul(acc[eb][:, 384:512], pk65_t[:, sl],
                                 pk65_t[:, E:E + 128], start=False,
                                 stop=False, skip_group_check=True)

        # main loop; the last chunk interleaves each block's finisher so the
        # epilogue of earlier blocks overlaps the remaining PE work
        for g in range(G):
            pt, pt4, pt5 = pre[g] if g < KPE else emit_pt(g)
            first = g == 0
            last = g == G - 1
            for eb in range(EB):
                ebo = eb * 128

                def pts(j):
                    return pt[:, j * E + ebo:j * E + ebo + 128]

                nc.tensor.matmul(acc[eb][:, 0:192], pts(1),
                                 wab_hi_t[:, bass.ts(g, 192)], start=first,
                                 stop=False, skip_group_check=True)
                nc.tensor.matmul(acc[eb][:, 0:192], pts(0),
                                 wab_lo_t[:, bass.ts(g, 192)], start=False,
                                 stop=last, skip_group_check=True)
                # NOTE: start=True marks the whole 2KB PSUM bank pending-zero,
                # so only the bank's very first matmul (ABhi at g=0) sets it;
                # other regions' first writes then overwrite-on-pending.
                for i in range(2):
                    nc.tensor.matmul(acc[eb][:, 192 + 64 * i:256 + 64 * i],
                                     pts(2 + i), wcc_t[:, bass.ts(g, 64)],
                                     start=False, stop=last,
                                     skip_group_check=True)
                if g % 2 == 1:
                    # fp8 DoubleRow: one matmul covers the chunk pair for
                    # both the C2 and D paths
                    gp = g // 2
                    nc.tensor.matmul(
                        acc[eb][:, 320:384],
                        pt4.rearrange(
                            "p (c e) -> p c e", c=2)[:, :, ebo:ebo + 128],
                        wc2_t[:, 128 * gp:128 * (gp + 1)].rearrange(
                            "p (c w) -> p c w", c=2),
                        start=False, stop=last, skip_group_check=True,
                        perf_mode=mybir.MatmulPerfMode.DoubleRow)
                    nc.tensor.matmul(
                        acc[eb][:, 384:512],
                        pt5.rearrange(
                            "p (c e) -> p c e", c=2)[:, :, ebo:ebo + 128],
                        wdd_t[:, 256 * gp:256 * (gp + 1)].rearrange(
                            "p (c w) -> p c w", c=2),
                        start=False, stop=last, skip_group_check=True,
                        perf_mode=mybir.MatmulPerfMode.DoubleRow)
                if last:
                    finish_eb(eb)
            if g == 10:
                emit_openers()
        for eb in range(EB):
            finish2_eb(eb)

    nc.compile()
    return nc


_NC = None


def _get_nc():
    global _NC
    if _NC is None:
        _NC = _build_nc()
    return _NC


def _prep_inputs(data_in1, data_in2, weight, W0, b0, W1, b1, W2, b2, bias):
    f32 = np.float32
    data_in1 = np.ascontiguousarray(data_in1, dtype=f32)
    data_in2 = np.ascontiguousarray(data_in2, dtype=f32)
    weight = np.ascontiguousarray(weight, dtype=f32)
    W0 = np.asarray(W0, f32); b0 = np.asarray(b0, f32)
    W1 = np.asarray(W1, f32); b1 = np.asarray(b1, f32)
    W2 = np.asarray(W2, f32); b2 = np.asarray(b2, f32)
    bias = np.asarray(bias, f32)

    s1 = data_in1[:, :MUL0]                      # [N,128]
    v1 = data_in1[:, MUL0:].reshape(N, MUL1, 3)  # [N,64,3]
    s2 = data_in2[:, 0]                          # [N]
    v2 = data_in2[:, 1:4]                        # [N,3]

    def bf(x):
        return np.ascontiguousarray(x, dtype=f32).astype(BF16_NP)

    s1t = s1.T                                   # [128,N] f32
    # fused KR x-operand blocks, each [128, N]
    s1lo = np.concatenate([s1t[0:64], s1t[0:64]], axis=0)
    s1hi = np.concatenate([s1t[64:128], s1t[64:128]], axis=0)
    vs = []
    for i in range(3):
        v1s2 = (v1[:, :, i] * s2[:, None]).T     # [64,N]
        vs.append(np.concatenate([v1s2, v1s2], axis=0))
    dot12 = np.einsum("eui,ei->eu", v1, v2).T    # [64,N]
    d2 = np.concatenate([dot12, dot12], axis=0)
    fin0 = bf(np.stack([s1lo, s1hi, vs[0], vs[1], vs[2], d2], axis=1))
    # fin0: [128, 6, N]
    d2o = bf(np.concatenate([dot12, np.ones((1, N), f32)], axis=0))  # [65,N]
    wT = bf(weight.T)

    # W2 chunk layouts: chunk g rows r=(koff*64+uu) -> W2x[2g+koff, sel(uu), :]
    def chunks(arr3, usel):  # arr3 [64,U,W] -> [128, G, W]
        a = arr3.reshape(G, 2, arr3.shape[1], arr3.shape[2])[:, :, usel, :]
        return np.transpose(a, (1, 2, 0, 3)).reshape(128, G, arr3.shape[2])

    Wa3 = W2[:, :N1].reshape(64, 128, 128)
    Wb3 = W2[:, N1:N1 + N2].reshape(64, 128, 64)
    Wc3 = W2[:, N1 + N2:N1 + N2 + N3].reshape(64, 64, 64)
    Wd3 = W2[:, N1 + N2 + N3:].reshape(64, 64, 128)
    lo, hi = slice(0, 64), slice(64, 128)

    def ab(usel):  # [128, G*192]: per chunk [Wa(128) | Wb(64)]
        return bf(np.concatenate(
            [chunks(Wa3, usel), chunks(Wb3, usel)], axis=2
        ).reshape(128, G * 192))

    # wg1K[m, 128g+r] = W1[m, 2g + r//64]: replicated W1 columns so the PE
    # broadcast for the first KPE chunks fuses layer 2 (works off h1)
    wg1K = np.repeat(W1, 64, axis=1)[:, :128 * KPE]
    # bg1K[p, g] = b1[2g + p//64]
    bg1K = np.concatenate(
        [np.broadcast_to(b1[0:2 * KPE:2], (64, KPE)),
         np.broadcast_to(b1[1:2 * KPE:2], (64, KPE))], axis=0).astype(f32)

    shared = {
        "pkw": bf(np.concatenate([W1, wg1K], axis=1)),
        "wab_lo": ab(lo),
        "wab_hi": ab(hi),
        "wcc": bf(chunks(Wc3, lo).reshape(128, G * 64)),
        "wc2": np.ascontiguousarray(
            chunks(Wc3, lo).reshape(128, G * 64), f32).astype(F8_NP),
        "wdd": np.ascontiguousarray(
            I3 * chunks(Wd3, lo).reshape(128, G * 128), f32).astype(F8_NP),
    }
    bab = np.concatenate(
        [b2[:N1].reshape(128, 128), b2[N1:N1 + N2].reshape(128, 64)], axis=1)
    bdb = np.concatenate(
        [I3 * b2[N1 + N2 + N3:].reshape(64, 128), bias[None, :]], axis=0)
    bc2p = np.concatenate(
        [b2[N1 + N2:N1 + N2 + N3].reshape(64, 64), np.zeros((1, 64), f32)],
        axis=0)

    in_maps = []
    for c in range(N_CORES):
        e0 = c * E
        m = dict(shared)
        m["fin0"] = np.ascontiguousarray(
            fin0[:, :, e0:e0 + E]).reshape(128, 6 * E)
        m["pk16"] = bf(np.concatenate([W0, wT[:, e0:e0 + E]], axis=1))
        m["pk65"] = bf(np.concatenate([d2o[:, e0:e0 + E], bdb, bc2p], axis=1))
        m["pk128"] = bf(np.concatenate(
            [s1t[:, e0:e0 + E], bab, np.eye(128, dtype=f32)], axis=1))
        # sv: cols 0:4 = s2 per sample block; then bg1K, then b0, b1
        sv = np.zeros((128, 6 + KPE), f32)
        sv[:, 4:4 + KPE] = bg1K
        sv[0:64, 4 + KPE] = b0
        sv[0:64, 5 + KPE] = b1
        v2b3 = np.zeros((128, EB, 3, 64), f32)
        for eb in range(EB):
            b0_ = e0 + eb * 128
            sv[:, eb] = s2[b0_:b0_ + 128]
            for i in range(3):
                v2b3[:, eb, i, :] = v2[b0_:b0_ + 128, i:i + 1]
        m["sv"] = sv
        m["v2b3"] = v2b3.reshape(128, EB * 192)
        in_maps.append(m)
    return in_maps


def run(in_maps, **kwargs):
    nc = _get_nc()
    return run_bass_kernel_spmd(nc, in_maps, list(range(N_CORES)), **kwargs)


def kernel(data_in1, data_in2, weight, W0, b0, W1, b1, W2, b2, bias):
    in_maps = _prep_inputs(
        data_in1, data_in2, weight, W0, b0, W1, b1, W2, b2, bias
    )
    res = run(in_maps)
    out = np.concatenate(
        [np.asarray(res.results[c]["outp"]) for c in range(N_CORES)], axis=0
    )
    return out.astype(np.float32)


# revision 54
# speedup vs baseline: 1.0086x; 1.0086x over previous
"""Trainium2 Bass kernel for nn_O3TensorProductWeighted.

Computes, for each sample e:
    h  = relu(relu(weight @ W0 + b0) @ W1 + b1)           # [64]
    w  = h @ W2 + b2                                      # [36864] (never materialized)
    out0 = PW0*(einsum(Wa,s1)*s2 + I3*einsum(Wd,dot12))
    out1 = PW1*I3*(einsum(Wb,s1) x v2 + einsum(Wc,v1)*s2)
    out  = concat(out0, out1)/SQRT_K ; out[:128] += bias

Strategy: reassociate each einsum against the (k,u)-joint contraction of the
per-sample Khatri-Rao product h (x) x, so everything becomes dense matmuls
over shared W2 chunk weights. Samples ride on PSUM partitions: each matmul
takes a [128 contraction, 128 samples] KR slice as the stationary and
streams W2 chunk columns as the moving operand, so PE cost equals true
output columns (bf16 roofline) and the result lands directly in [E, 320]
layout - no final transposes. The D path (dot12 x h against Wd, ~36% of the
output norm) runs as fp8-e4m3 DoubleRow matmuls covering a chunk pair per
instruction at the doubled fp8 PE rate, adding ~1.1e-2 rel err against the
2e-2 gate. The per-chunk h-broadcast is a partition-replicating SBUF->SBUF
DMA from h2 (for the first KPE chunks, a PE matmul against replicated W1
columns straight from h1, skipping the h2 dependency); the KR multiply is
split DVE (blocks 0-3, bf16 2x) / Pool (block 4 bf16 + block 5 fp8), with
blocks 2-3 shifting to Pool for the final chunks so both engines drain
together. b2/bias opener matmuls run mid-loop (accumulation is commutative),
keeping their params off the startup critical path and the openers off the
tail; the output bias row rides a ones contraction row in the D opener.
Epilogue: s2/v2 are per-partition scalars -> ACT scale + fused DVE adds per
128-sample block, interleaved with the last chunk's matmuls. Weight slabs
stream on the SP ring in consumption order; fin0 loads via Pool SWDGE; the
ACT ring carries the broadcast DMAs. Pure data parallel over 8 cores, 512
samples each.
"""

import dataclasses
import sys

sys.path.insert(0, "/opt/trn_rl_repo")

from contextlib import ExitStack

import ml_dtypes
import numpy as np

import concourse.bacc as bacc
import concourse.bass as bass
import concourse.tile as tile
from concourse import mybir
from concourse.bass_utils import run_bass_kernel_spmd

BF16 = mybir.dt.bfloat16
F32 = mybir.dt.float32
F8 = mybir.dt.float8e4
BF16_NP = ml_dtypes.bfloat16
F8_NP = ml_dtypes.float8_e4m3fn

N_CORES = 8
N = 4096
E = N // N_CORES  # 512 samples per core
EB = E // 128     # 4 sample blocks of 128 (PSUM partition dim)

MUL0, MUL1 = 128, 64
N1 = MUL0 * MUL0          # 16384
N2 = MUL0 * MUL1          # 8192
N3 = MUL1 * MUL1          # 4096
I3 = float(1.0 / np.sqrt(3.0))
# PW0/SQRT_K == 1.0 and PW1*I3/SQRT_K == 1.0 exactly; only I3 remains on D,
# folded into the wdd/bdb host tensors.

G = 32   # chunks; chunk g covers k in {2g, 2g+1} x 64 u-values (128 rows)
KPE = 5  # chunks whose h-broadcast runs on PE while the DMA path warms up


def _build_nc():
    nc = bacc.Bacc(None)

    def dram(name, shape, dt=BF16):
        return nc.declare_dram_parameter(name, shape, dt, isOutput=False)

    # per-core inputs
    # fused KR x-operand: [s1lo2 | s1hi2 | vs0 | vs1 | vs2 | d22], each [128, E]
    fin0_d = dram("fin0", [128, 6 * E])
    pk16_d = dram("pk16", [16, 64 + E])        # [w0 | wT]
    pkw_d = dram("pkw", [64, 64 + 128 * KPE])  # [w1 | wg1K]
    pk65_d = dram("pk65", [65, E + 192])       # [d2o | bdb | bc2(64 rows)]
    pk128_d = dram("pk128", [128, E + 320])    # [s1t | bab | ident]
    sv_d = dram("sv", [128, 6 + KPE], F32)  # [s2 per block | bg1K | b0 | b1]
    v2b3_d = dram("v2b3", [128, EB * 192], F32)  # v2_i bcast per sample block

    # shared W2 chunk tensors; the D path runs in fp8 DoubleRow (its term is
    # ~36% of the output norm, adding ~1.1e-2 rel err - inside the 2e-2 gate)
    wab_lo_d = dram("wab_lo", [128, G * 192])
    wab_hi_d = dram("wab_hi", [128, G * 192])
    wcc_d = dram("wcc", [128, G * 64])
    wc2_d = dram("wc2", [128, G * 64], F8)
    wdd_d = dram("wdd", [128, G * 128], F8)

    outp_d = nc.declare_dram_parameter("outp", [E, 320], F32, isOutput=True)

    with tile.TileContext(nc) as tc, ExitStack() as ctx:
        const = ctx.enter_context(tc.tile_pool(name="const", bufs=1))
        work = ctx.enter_context(tc.tile_pool(name="work", bufs=1))
        bct_pool = ctx.enter_context(tc.tile_pool(name="bct", bufs=KPE + 2))
        pt_pool = ctx.enter_context(tc.tile_pool(name="pt", bufs=KPE + 2))
        pt5_pool = ctx.enter_context(tc.tile_pool(name="pt5", bufs=4))
        out_pool = ctx.enter_context(tc.tile_pool(name="outs", bufs=4))
        ps_acc = ctx.enter_context(tc.tile_pool(name="ps_acc", bufs=1, space="PSUM"))
        ps_mlp = ctx.enter_context(tc.tile_pool(name="ps_mlp", bufs=1, space="PSUM"))
        ps_bc = ctx.enter_context(tc.tile_pool(name="ps_bc", bufs=2, space="PSUM"))

        def tileof(dparam):
            return const.tile(dparam.shape, dparam.dtype, name=f"t_{dparam.name}")

        # sv (scales + all MLP biases) rides the idle ACT ring at t=0 so the
        # startup chain never waits on the SP ring behind pk16/pkw.
        sv_t = tileof(sv_d)
        nc.scalar.dma_start(sv_t[:], sv_d[:])
        # SP ring: MLP/broadcast params first, then W2 slabs in consumption
        # order (fine slabs first so chunk 0 weights land just in time).
        pk16_t = tileof(pk16_d)
        nc.sync.dma_start(pk16_t[:, 0:320], pk16_d[:, 0:320])
        nc.sync.dma_start(pk16_t[:, 320:64 + E], pk16_d[:, 320:64 + E])
        pkw_t = tileof(pkw_d)
        nc.sync.dma_start(pkw_t[:], pkw_d[:])
        wab_hi_t = tileof(wab_hi_d)
        wab_lo_t = tileof(wab_lo_d)
        wcc_t = tileof(wcc_d)
        wc2_t = tileof(wc2_d)
        wdd_t = tileof(wdd_d)
        slabs = [(0, 4), (4, 8), (8, 16), (16, 24), (24, 32)]
        for si, (g0, g1) in enumerate(slabs):
            for t, dp, w in ((wab_hi_t, wab_hi_d, 192), (wab_lo_t, wab_lo_d, 192),
                             (wcc_t, wcc_d, 64), (wc2_t, wc2_d, 64),
                             (wdd_t, wdd_d, 128)):
                nc.sync.dma_start(t[:, g0 * w:g1 * w], dp[:, g0 * w:g1 * w])
            if si == 1:
                pk128_t = tileof(pk128_d)
                nc.sync.dma_start(pk128_t[:], pk128_d[:])
            elif si == 2:
                pk65_t = tileof(pk65_d)
                nc.sync.dma_start(pk65_t[:], pk65_d[:])
            elif si == 3:
                v2b3_t = tileof(v2b3_d)
                nc.sync.dma_start(v2b3_t[:], v2b3_d[:])

        # Pool self-loads fin0 via SWDGE before its multiply work begins
        fin0_t = tileof(fin0_d)
        nc.gpsimd.dma_start(fin0_t[:, 4 * E:6 * E], fin0_d[:, 4 * E:6 * E])
        nc.gpsimd.dma_start(fin0_t[:, 0:4 * E], fin0_d[:, 0:4 * E])

        # ACT warmup: pay the activation-table load before the MLP needs ACT
        warm = work.tile([1, 8], F32, name="warm")
        nc.vector.memset(warm[:], 0.0)
        nc.scalar.activation(warm[:], warm[:],
                             mybir.ActivationFunctionType.Relu, bias=0.0,
                             scale=1.0)

        # MLP layer 1: h1 = relu(W0.T @ wT + b0) : [64, E], E-split (with
        # separate PSUM tiles, avoiding a false WAR between the halves) so
        # the chunk-0 chain can proceed on the first half early
        h1_t = work.tile([64, E], BF16, name="h1")
        ps_h1 = ps_mlp.tile([128, 256], F32, name="ps_h1")
        for hi, (c0, c1) in enumerate(((0, 256), (256, E))):
            half = ps_h1[64 * hi:64 * (hi + 1), :]
            nc.tensor.matmul(half, pk16_t[:, 0:64],
                             pk16_t[:, 64 + c0:64 + c1], start=True, stop=True,
                             skip_group_check=True)
            nc.scalar.activation(h1_t[:, c0:c1], half,
                                 mybir.ActivationFunctionType.Relu,
                                 bias=sv_t[0:64, 4 + KPE:5 + KPE], scale=1.0)

        # persistent PSUM accumulators: one bank per 128-sample block,
        # columns [A(128) | B(64) | C0 C1 C2 (64 each) | D(128)]
        acc = [ps_acc.tile([128, 512], F32, name=f"acc{eb}") for eb in range(EB)]

        h2_t = work.tile([64, E], BF16, name="h2")

        # per-chunk KR product. First KPE chunks fuse layer 2 into a PE
        # replicated-W1-column matmul on h1 (no h2 dependency); later chunks
        # use a partition-replicating DMA from h2 on the ACT ring. Blocks 0-3
        # multiply on DVE (bf16, 2x), block 4 on Pool (bf16), block 5 on Pool
        # into the fp8 chunk-pair tile feeding the DoubleRow D matmuls.
        pt45_cur = [None]

        def emit_pt(g):
            bct = bct_pool.tile([128, E], BF16, tag="bct")
            if g < KPE:
                psb = ps_bc.tile([128, E], F32, tag="psb")
                halves = ((0, 256), (256, E)) if g == 0 else ((0, E),)
                for c0, c1 in halves:
                    nc.tensor.matmul(psb[:, c0:c1],
                                     pkw_t[:, 64 + 128 * g:64 + 128 * (g + 1)],
                                     h1_t[:, c0:c1], start=c0 == 0,
                                     stop=c1 == E, skip_group_check=True)
                    if g == 0 and c0 == 0:
                        # DVE computes the first quarter's relu itself so its
                        # stream starts without waiting on the ACT queue
                        nc.vector.tensor_scalar(
                            bct[:, 0:256], psb[:, 0:256],
                            sv_t[:, 4 + g:5 + g], 0.0,
                            mybir.AluOpType.add, mybir.AluOpType.max)
                    else:
                        nc.scalar.activation(bct[:, c0:c1], psb[:, c0:c1],
                                             mybir.ActivationFunctionType.Relu,
                                             bias=sv_t[:, 4 + g:5 + g],
                                             scale=1.0)
            else:
                src = h2_t[2 * g:2 * g + 2, :]
                src_b = dataclasses.replace(src, ap=[src.ap[0], [0, 64], [1, E]])
                nc.scalar.dma_start(bct[:], src_b)

            pt = pt_pool.tile([128, 4 * E], BF16, tag="pt")
            # chunk 0's DVE multiply is split so the AB matmuls (blocks 0/1)
            # can start while blocks 2/3 are still being produced. Pool runs
            # ~200ns/chunk ahead of DVE, so for the last 4 chunks blocks 2-3
            # shift to Pool and both engines finish the KR stream early.
            if g == 0:
                dve_parts, pool_parts = (), ()
                for c0, c1 in ((0, 256), (256, E)):
                    for b0_, b1_ in ((0, 2), (2, 4)):
                        nb = b1_ - b0_
                        bctb = dataclasses.replace(
                            bct[:, c0:c1],
                            ap=[bct[:, c0:c1].ap[0], [0, nb], [1, c1 - c0]])
                        nc.vector.tensor_mul(
                            pt[:, b0_ * E:b1_ * E].rearrange(
                                "p (b e) -> p b e", b=nb)[:, :, c0:c1],
                            fin0_t[:, b0_ * E:b1_ * E].rearrange(
                                "p (b e) -> p b e", b=nb)[:, :, c0:c1], bctb)
            elif g >= G - 5:
                dve_parts, pool_parts = ((0, 2),), ((2, 4),)
            else:
                dve_parts, pool_parts = ((0, 4),), ()
            for b0_, b1_ in dve_parts:
                nb = b1_ - b0_
                bctb = dataclasses.replace(
                    bct[:], ap=[bct[:].ap[0], [0, nb], [1, E]])
                nc.vector.tensor_mul(
                    pt[:, b0_ * E:b1_ * E].rearrange("p (b e) -> p b e", b=nb),
                    fin0_t[:, b0_ * E:b1_ * E].rearrange(
                        "p (b e) -> p b e", b=nb), bctb)
            for b0_, b1_ in pool_parts:
                if b0_ == 4:
                    continue
                nb = b1_ - b0_
                bctb = dataclasses.replace(
                    bct[:], ap=[bct[:].ap[0], [0, nb], [1, E]])
                nc.gpsimd.tensor_mul(
                    pt[:, b0_ * E:b1_ * E].rearrange("p (b e) -> p b e", b=nb),
                    fin0_t[:, b0_ * E:b1_ * E].rearrange(
                        "p (b e) -> p b e", b=nb), bctb)
            if g % 2 == 0:
                pt45_cur[0] = pt5_pool.tile([128, 4 * E], F8, tag="pt45",
                                            name="pt45")
            ptf = pt45_cur[0]
            dst = dataclasses.replace(
                ptf[:, (g % 2) * E:(g % 2 + 1) * E],
                ap=[ptf[:].ap[0], [2 * E, 2], [1, E]])
            bct2 = dataclasses.replace(
                bct[:], ap=[bct[:].ap[0], [0, 2], [1, E]])
            nc.gpsimd.tensor_mul(
                dst, fin0_t[:, 4 * E:6 * E].rearrange("p (b e) -> p b e", b=2),
                bct2)
            return pt, ptf[:, 0:2 * E], ptf[:, 2 * E:4 * E]

        # prefill so PE never waits on the broadcast/multiply chain mid-loop;
        # layer 2 (h2, feeding the DMA-replicated chunks) is slotted between
        # the first prefill chunks so it stays off the critical path.
        pre = [emit_pt(0), emit_pt(1)]
        ps_h2 = ps_mlp.tile([64, E], F32, name="ps_h2")
        nc.tensor.matmul(ps_h2[:], pkw_t[:, 0:64], h1_t[:], start=True, stop=True)
        nc.scalar.activation(h2_t[:], ps_h2[:], mybir.ActivationFunctionType.Relu,
                             bias=sv_t[0:64, 5 + KPE:6 + KPE], scale=1.0)
        pre += [emit_pt(g) for g in range(2, KPE)]

        # per sample block finisher: openers (b2 bias terms + output bias via
        # the ones row; accumulation is commutative so they close each group),
        # then the ACT/DVE epilogue:
        # out0 = A*s2 + (I3*D + bias) ; out1_i = B*v2_i + C_i
        mult_op = mybir.AluOpType.mult
        add_op = mybir.AluOpType.add

        tAs, tBs = [], []

        def finish_eb(eb):
            # phase 1: per-partition scale factors into bf16 staging tiles
            tA = work.tile([128, 128], BF16, tag=f"tA{eb}", name=f"tA{eb}")
            nc.scalar.mul(tA[:], acc[eb][:, 0:128], sv_t[:, eb:eb + 1])
            tB = work.tile([128, 192], BF16, tag=f"tB{eb}", name=f"tB{eb}")
            accB = acc[eb][:, 128:192]
            accB3 = dataclasses.replace(accB, ap=[accB.ap[0], [0, 3], [1, 64]])
            nc.vector.tensor_mul(
                tB[:].rearrange("p (i w) -> p i w", i=3),
                v2b3_t[:, eb * 192:(eb + 1) * 192].rearrange(
                    "p (i w) -> p i w", i=3), accB3)
            tAs.append(tA); tBs.append(tB)

        def finish2_eb(eb):
            # phase 2: identity matmuls add the scaled terms into the PSUM
            # accumulators (PE is cheap: cost = output columns), then plain
            # ACT/DVE evacuations feed the output DMA
            sl = bass.ts(eb, 128)
            ident = pk128_t[:, E + 192:E + 320]
            nc.tensor.matmul(acc[eb][:, 384:512], ident, tAs[eb][:],
                             start=False, stop=False, skip_group_check=True)
            nc.tensor.matmul(acc[eb][:, 192:384], ident, tBs[eb][:],
                             start=False, stop=False, skip_group_check=True)
            outS = out_pool.tile([128, 320], F32, tag="outS")
            nc.scalar.copy(outS[:, 0:128], acc[eb][:, 384:512])
            o1v = outS[:, 128:320].rearrange("p (w i) -> p i w", i=3)
            nc.vector.tensor_copy(
                o1v, acc[eb][:, 192:384].rearrange("p (i w) -> p i w", i=3))
            (nc.sync if eb % 2 == 0 else nc.scalar).dma_start(
                outp_d[sl, :], outS[:])

        def emit_openers():
            # b2 bias terms + output bias (ones row): accumulation is
            # commutative, so these run mid-loop once their params are loaded
            for eb in range(EB):
                sl = bass.ts(eb, 128)
                nc.tensor.matmul(acc[eb][:, 0:192], pk128_t[:, sl],
                                 pk128_t[:, E:E + 192], start=False,
                                 stop=False, skip_group_check=True)
                for i in range(3):
                    nc.tensor.matmul(acc[eb][:, 192 + 64 * i:256 + 64 * i],
                                     fin0_t[0:64, (2 + i) * E + eb * 128:
                                            (2 + i) * E + eb * 128 + 128],
                                     pk65_t[0:64, E + 128:E + 192],
                                     start=False, stop=False,
                                     skip_group_check=True)
                nc.tensor.matmul(acc[eb][:, 384:512], pk65_t[:, sl],
                                 pk65_t[:, E:E + 128], start=False,
                                 stop=False, skip_group_check=True)

        # main loop; the last chunk interleaves each block's finisher so the
        # epilogue of earlier blocks overlaps the remaining PE work
        for g in range(G):
            pt, pt4, pt5 = pre[g] if g < KPE else emit_pt(g)
            first = g == 0
            last = g == G - 1
            for eb in range(EB):
                ebo = eb * 128

                def pts(j):
                    return pt[:, j * E + ebo:j * E + ebo + 128]

                nc.tensor.matmul(acc[eb][:, 0:192], pts(1),
                                 wab_hi_t[:, bass.ts(g, 192)], start=first,
                                 stop=False, skip_group_check=True)
                nc.tensor.matmul(acc[eb][:, 0:192], pts(0),
                                 wab_lo_t[:, bass.ts(g, 192)], start=False,
                                 stop=last, skip_group_check=True)
                # NOTE: start=True marks the whole 2KB PSUM bank pending-zero,
                # so only the bank's very first matmul (ABhi at g=0) sets it;
                # other regions' first writes then overwrite-on-pending.
                for i in range(2):
                    nc.tensor.matmul(acc[eb][:, 192 + 64 * i:256 + 64 * i],
                                     pts(2 + i), wcc_t[:, bass.ts(g, 64)],
                                     start=False, stop=last,
                                     skip_group_check=True)
                if g % 2 == 1:
                    # fp8 DoubleRow: one matmul covers the chunk pair for
                    # both the C2 and D paths
                    gp = g // 2
                    nc.tensor.matmul(
                        acc[eb][:, 320:384],
                        pt4.rearrange(
                            "p (c e) -> p c e", c=2)[:, :, ebo:ebo + 128],
                        wc2_t[:, 128 * gp:128 * (gp + 1)].rearrange(
                            "p (c w) -> p c w", c=2),
                        start=False, stop=last, skip_group_check=True,
                        perf_mode=mybir.MatmulPerfMode.DoubleRow)
                    nc.tensor.matmul(
                        acc[eb][:, 384:512],
                        pt5.rearrange(
                            "p (c e) -> p c e", c=2)[:, :, ebo:ebo + 128],
                        wdd_t[:, 256 * gp:256 * (gp + 1)].rearrange(
                            "p (c w) -> p c w", c=2),
                        start=False, stop=last, skip_group_check=True,
                        perf_mode=mybir.MatmulPerfMode.DoubleRow)
                if last:
                    finish_eb(eb)
            if g == 10:
                emit_openers()
        for eb in range(EB):
            finish2_eb(eb)

    nc.compile()
    return nc


_NC = None


def _get_nc():
    global _NC
    if _NC is None:
        _NC = _build_nc()
    return _NC


def _prep_inputs(data_in1, data_in2, weight, W0, b0, W1, b1, W2, b2, bias):
    f32 = np.float32
    data_in1 = np.ascontiguousarray(data_in1, dtype=f32)
    data_in2 = np.ascontiguousarray(data_in2, dtype=f32)
    weight = np.ascontiguousarray(weight, dtype=f32)
    W0 = np.asarray(W0, f32); b0 = np.asarray(b0, f32)
    W1 = np.asarray(W1, f32); b1 = np.asarray(b1, f32)
    W2 = np.asarray(W2, f32); b2 = np.asarray(b2, f32)
    bias = np.asarray(bias, f32)

    s1 = data_in1[:, :MUL0]                      # [N,128]
    v1 = data_in1[:, MUL0:].reshape(N, MUL1, 3)  # [N,64,3]
    s2 = data_in2[:, 0]                          # [N]
    v2 = data_in2[:, 1:4]                        # [N,3]

    def bf(x):
        return np.ascontiguousarray(x, dtype=f32).astype(BF16_NP)

    s1t = s1.T                                   # [128,N] f32
    # fused KR x-operand blocks, each [128, N]
    s1lo = np.concatenate([s1t[0:64], s1t[0:64]], axis=0)
    s1hi = np.concatenate([s1t[64:128], s1t[64:128]], axis=0)
    vs = []
    for i in range(3):
        v1s2 = (v1[:, :, i] * s2[:, None]).T     # [64,N]
        vs.append(np.concatenate([v1s2, v1s2], axis=0))
    dot12 = np.einsum("eui,ei->eu", v1, v2).T    # [64,N]
    d2 = np.concatenate([dot12, dot12], axis=0)
    fin0 = bf(np.stack([s1lo, s1hi, vs[0], vs[1], vs[2], d2], axis=1))
    # fin0: [128, 6, N]
    d2o = bf(np.concatenate([dot12, np.ones((1, N), f32)], axis=0))  # [65,N]
    wT = bf(weight.T)

    # W2 chunk layouts: chunk g rows r=(koff*64+uu) -> W2x[2g+koff, sel(uu), :]
    def chunks(arr3, usel):  # arr3 [64,U,W] -> [128, G, W]
        a = arr3.reshape(G, 2, arr3.shape[1], arr3.shape[2])[:, :, usel, :]
        return np.transpose(a, (1, 2, 0, 3)).reshape(128, G, arr3.shape[2])

    Wa3 = W2[:, :N1].reshape(64, 128, 128)
    Wb3 = W2[:, N1:N1 + N2].reshape(64, 128, 64)
    Wc3 = W2[:, N1 + N2:N1 + N2 + N3].reshape(64, 64, 64)
    Wd3 = W2[:, N1 + N2 + N3:].reshape(64, 64, 128)
    lo, hi = slice(0, 64), slice(64, 128)

    def ab(usel):  # [128, G*192]: per chunk [Wa(128) | Wb(64)]
        return bf(np.concatenate(
            [chunks(Wa3, usel), chunks(Wb3, usel)], axis=2
        ).reshape(128, G * 192))

    # wg1K[m, 128g+r] = W1[m, 2g + r//64]: replicated W1 columns so the PE
    # broadcast for the first KPE chunks fuses layer 2 (works off h1)
    wg1K = np.repeat(W1, 64, axis=1)[:, :128 * KPE]
    # bg1K[p, g] = b1[2g + p//64]
    bg1K = np.concatenate(
        [np.broadcast_to(b1[0:2 * KPE:2], (64, KPE)),
         np.broadcast_to(b1[1:2 * KPE:2], (64, KPE))], axis=0).astype(f32)

    shared = {
        "pkw": bf(np.concatenate([W1, wg1K], axis=1)),
        "wab_lo": ab(lo),
        "wab_hi": ab(hi),
        "wcc": bf(chunks(Wc3, lo).reshape(128, G * 64)),
        "wc2": np.ascontiguousarray(
            chunks(Wc3, lo).reshape(128, G * 64), f32).astype(F8_NP),
        "wdd": np.ascontiguousarray(
            I3 * chunks(Wd3, lo).reshape(128, G * 128), f32).astype(F8_NP),
    }
    bab = np.concatenate(
        [b2[:N1].reshape(128, 128), b2[N1:N1 + N2].reshape(128, 64)], axis=1)
    bdb = np.concatenate(
        [I3 * b2[N1 + N2 + N3:].reshape(64, 128), bias[None, :]], axis=0)
    bc2p = np.concatenate(
        [b2[N1 + N2:N1 + N2 + N3].reshape(64, 64), np.zeros((1, 64), f32)],
        axis=0)

    in_maps = []
    for c in range(N_CORES):
        e0 = c * E
        m = dict(shared)
        m["fin0"] = np.ascontiguousarray(
            fin0[:, :, e0:e0 + E]).reshape(128, 6 * E)
        m["pk16"] = bf(np.concatenate([W0, wT[:, e0:e0 + E]], axis=1))
        m["pk65"] = bf(np.concatenate([d2o[:, e0:e0 + E], bdb, bc2p], axis=1))
        m["pk128"] = bf(np.concatenate(
            [s1t[:, e0:e0 + E], bab, np.eye(128, dtype=f32)], axis=1))
        # sv: cols 0:4 = s2 per sample block; then bg1K, then b0, b1
        sv = np.zeros((128, 6 + KPE), f32)
        sv[:, 4:4 + KPE] = bg1K
        sv[0:64, 4 + KPE] = b0
        sv[0:64, 5 + KPE] = b1
        v2b3 = np.zeros((128, EB, 3, 64), f32)
        for eb in range(EB):
            b0_ = e0 + eb * 128
            sv[:, eb] = s2[b0_:b0_ + 128]
            for i in range(3):
                v2b3[:, eb, i, :] = v2[b0_:b0_ + 128, i:i + 1]
        m["sv"] = sv
        m["v2b3"] = v2b3.reshape(128, EB * 192)
        in_maps.append(m)
    return in_maps


def run(in_maps, **kwargs):
    nc = _get_nc()
    return run_bass_kernel_spmd(nc, in_maps, list(range(N_CORES)), **kwargs)


def kernel(data_in1, data_in2, weight, W0, b0, W1, b1, W2, b2, bias):
    in_maps = _prep_inputs(
        data_in1, data_in2, weight, W0, b0, W1, b1, W2, b2, bias
    )
    res = run(in_maps)
    out = np.concatenate(
        [np.asarray(res.results[c]["outp"]) for c in range(N_CORES)], axis=0
    )
    return out.astype(np.float32)


# revision 58
# speedup vs baseline: 1.0116x; 1.0029x over previous
"""Trainium2 Bass kernel for nn_O3TensorProductWeighted.

Computes, for each sample e:
    h  = relu(relu(weight @ W0 + b0) @ W1 + b1)           # [64]
    w  = h @ W2 + b2                                      # [36864] (never materialized)
    out0 = PW0*(einsum(Wa,s1)*s2 + I3*einsum(Wd,dot12))
    out1 = PW1*I3*(einsum(Wb,s1) x v2 + einsum(Wc,v1)*s2)
    out  = concat(out0, out1)/SQRT_K ; out[:128] += bias

Strategy: reassociate each einsum against the (k,u)-joint contraction of the
per-sample Khatri-Rao product h (x) x, so everything becomes dense matmuls
over shared W2 chunk weights. Samples ride on PSUM partitions: each matmul
takes a [128 contraction, 128 samples] KR slice as the stationary and
streams W2 chunk columns as the moving operand, so PE cost equals true
output columns (bf16 roofline) and the result lands directly in [E, 320]
layout - no final transposes. The D path (dot12 x h against Wd, ~36% of the
output norm) runs as fp8-e4m3 DoubleRow matmuls covering a chunk pair per
instruction at the doubled fp8 PE rate, adding ~1.1e-2 rel err against the
2e-2 gate. The per-chunk h-broadcast is a partition-replicating SBUF->SBUF
DMA from h2 (for the first KPE chunks, a PE matmul against replicated W1
columns straight from h1, skipping the h2 dependency); the KR multiply is
split DVE (blocks 0-3, bf16 2x) / Pool (block 4 bf16 + block 5 fp8), with
blocks 2-3 shifting to Pool for the final chunks so both engines drain
together. b2/bias opener matmuls run mid-loop (accumulation is commutative),
keeping their params off the startup critical path and the openers off the
tail; the output bias row rides a ones contraction row in the D opener.
Epilogue: s2/v2 are per-partition scalars -> ACT scale + fused DVE adds per
128-sample block, interleaved with the last chunk's matmuls. Weight slabs
stream on the SP ring in consumption order; fin0 loads via Pool SWDGE; the
ACT ring carries the broadcast DMAs. Pure data parallel over 8 cores, 512
samples each.
"""

import dataclasses
import sys

sys.path.insert(0, "/opt/trn_rl_repo")

from contextlib import ExitStack

import ml_dtypes
import numpy as np

import concourse.bacc as bacc
import concourse.bass as bass
import concourse.tile as tile
from concourse import mybir
from concourse.bass_utils import run_bass_kernel_spmd

BF16 = mybir.dt.bfloat16
F32 = mybir.dt.float32
F8 = mybir.dt.float8e4
BF16_NP = ml_dtypes.bfloat16
F8_NP = ml_dtypes.float8_e4m3fn

N_CORES = 8
N = 4096
E = N // N_CORES  # 512 samples per core
EB = E // 128     # 4 sample blocks of 128 (PSUM partition dim)

MUL0, MUL1 = 128, 64
N1 = MUL0 * MUL0          # 16384
N2 = MUL0 * MUL1          # 8192
N3 = MUL1 * MUL1          # 4096
I3 = float(1.0 / np.sqrt(3.0))
# PW0/SQRT_K == 1.0 and PW1*I3/SQRT_K == 1.0 exactly; only I3 remains on D,
# folded into the wdd/bdb host tensors.

G = 32   # chunks; chunk g covers k in {2g, 2g+1} x 64 u-values (128 rows)
KPE = 5  # chunks whose h-broadcast runs on PE while the DMA path warms up


def _build_nc():
    nc = bacc.Bacc(None)

    def dram(name, shape, dt=BF16):
        return nc.declare_dram_parameter(name, shape, dt, isOutput=False)

    # per-core inputs
    # fused KR x-operand: [s1lo2 | s1hi2 | vs0 | vs1 | vs2 | d22], each [128, E]
    fin0_d = dram("fin0", [128, 6 * E])
    pk16_d = dram("pk16", [16, 64 + E])        # [w0 | wT]
    pkw_d = dram("pkw", [64, 64 + 128 * KPE])  # [w1 | wg1K]
    pk65_d = dram("pk65", [65, E + 192])       # [d2o | bdb | bc2(64 rows)]
    pk128_d = dram("pk128", [128, E + 320])    # [s1t | bab | ident]
    sv_d = dram("sv", [128, 6 + KPE], F32)  # [s2 per block | bg1K | b0 | b1]
    v2b3_d = dram("v2b3", [128, EB * 192], F32)  # v2_i bcast per sample block

    # shared W2 chunk tensors; the D path runs in fp8 DoubleRow (its term is
    # ~36% of the output norm, adding ~1.1e-2 rel err - inside the 2e-2 gate)
    wab_lo_d = dram("wab_lo", [128, G * 192])
    wab_hi_d = dram("wab_hi", [128, G * 192])
    wcc_d = dram("wcc", [128, G * 64])
    wc2_d = dram("wc2", [128, G * 64], F8)
    wdd_d = dram("wdd", [128, G * 128], F8)

    outp_d = nc.declare_dram_parameter("outp", [E, 320], F32, isOutput=True)

    with tile.TileContext(nc) as tc, ExitStack() as ctx:
        const = ctx.enter_context(tc.tile_pool(name="const", bufs=1))
        work = ctx.enter_context(tc.tile_pool(name="work", bufs=1))
        bct_pool = ctx.enter_context(tc.tile_pool(name="bct", bufs=KPE + 2))
        pt_pool = ctx.enter_context(tc.tile_pool(name="pt", bufs=KPE + 2))
        pt5_pool = ctx.enter_context(tc.tile_pool(name="pt5", bufs=4))
        out_pool = ctx.enter_context(tc.tile_pool(name="outs", bufs=4))
        ps_acc = ctx.enter_context(tc.tile_pool(name="ps_acc", bufs=1, space="PSUM"))
        ps_mlp = ctx.enter_context(tc.tile_pool(name="ps_mlp", bufs=1, space="PSUM"))
        ps_bc = ctx.enter_context(tc.tile_pool(name="ps_bc", bufs=2, space="PSUM"))

        def tileof(dparam):
            return const.tile(dparam.shape, dparam.dtype, name=f"t_{dparam.name}")

        # sv (scales + all MLP biases) rides the idle ACT ring at t=0 so the
        # startup chain never waits on the SP ring behind pk16/pkw.
        sv_t = tileof(sv_d)
        nc.scalar.dma_start(sv_t[:], sv_d[:])
        # SP ring: MLP/broadcast params first, then W2 slabs in consumption
        # order (fine slabs first so chunk 0 weights land just in time).
        pk16_t = tileof(pk16_d)
        nc.sync.dma_start(pk16_t[:, 0:320], pk16_d[:, 0:320])
        nc.sync.dma_start(pk16_t[:, 320:64 + E], pk16_d[:, 320:64 + E])
        pkw_t = tileof(pkw_d)
        nc.sync.dma_start(pkw_t[:], pkw_d[:])
        wab_hi_t = tileof(wab_hi_d)
        wab_lo_t = tileof(wab_lo_d)
        wcc_t = tileof(wcc_d)
        wc2_t = tileof(wc2_d)
        wdd_t = tileof(wdd_d)
        slabs = [(0, 4), (4, 8), (8, 16), (16, 24), (24, 32)]
        for si, (g0, g1) in enumerate(slabs):
            for t, dp, w in ((wab_hi_t, wab_hi_d, 192), (wab_lo_t, wab_lo_d, 192),
                             (wcc_t, wcc_d, 64), (wc2_t, wc2_d, 64),
                             (wdd_t, wdd_d, 128)):
                nc.sync.dma_start(t[:, g0 * w:g1 * w], dp[:, g0 * w:g1 * w])
            if si == 1:
                pk128_t = tileof(pk128_d)
                nc.sync.dma_start(pk128_t[:], pk128_d[:])
            elif si == 2:
                pk65_t = tileof(pk65_d)
                nc.sync.dma_start(pk65_t[:], pk65_d[:])
            elif si == 3:
                v2b3_t = tileof(v2b3_d)
                nc.sync.dma_start(v2b3_t[:], v2b3_d[:])

        # Pool self-loads fin0 via SWDGE before its multiply work begins
        fin0_t = tileof(fin0_d)
        nc.gpsimd.dma_start(fin0_t[:, 4 * E:6 * E], fin0_d[:, 4 * E:6 * E])
        nc.gpsimd.dma_start(fin0_t[:, 0:4 * E], fin0_d[:, 0:4 * E])

        # ACT warmup: pay the activation-table load before the MLP needs ACT
        warm = work.tile([1, 8], F32, name="warm")
        nc.vector.memset(warm[:], 0.0)
        nc.scalar.activation(warm[:], warm[:],
                             mybir.ActivationFunctionType.Relu, bias=0.0,
                             scale=1.0)

        # MLP layer 1: h1 = relu(W0.T @ wT + b0) : [64, E], E-split (with
        # separate PSUM tiles, avoiding a false WAR between the halves) so
        # the chunk-0 chain can proceed on the first half early
        h1_t = work.tile([64, E], BF16, name="h1")
        ps_h1 = ps_mlp.tile([128, 256], F32, name="ps_h1")
        for hi, (c0, c1) in enumerate(((0, 256), (256, E))):
            half = ps_h1[64 * hi:64 * (hi + 1), :]
            nc.tensor.matmul(half, pk16_t[:, 0:64],
                             pk16_t[:, 64 + c0:64 + c1], start=True, stop=True,
                             skip_group_check=True)
            nc.scalar.activation(h1_t[:, c0:c1], half,
                                 mybir.ActivationFunctionType.Relu,
                                 bias=sv_t[0:64, 4 + KPE:5 + KPE], scale=1.0)

        # persistent PSUM accumulators: one bank per 128-sample block,
        # columns [A(128) | B(64) | C0 C1 C2 (64 each) | D(128)]
        acc = [ps_acc.tile([128, 512], F32, name=f"acc{eb}") for eb in range(EB)]

        h2_t = work.tile([64, E], BF16, name="h2")

        # per-chunk KR product. First KPE chunks fuse layer 2 into a PE
        # replicated-W1-column matmul on h1 (no h2 dependency); later chunks
        # use a partition-replicating DMA from h2 on the ACT ring. Blocks 0-3
        # multiply on DVE (bf16, 2x), block 4 on Pool (bf16), block 5 on Pool
        # into the fp8 chunk-pair tile feeding the DoubleRow D matmuls.
        pt45_cur = [None]

        def emit_pt(g):
            bct = bct_pool.tile([128, E], BF16, tag="bct")
            if g < KPE:
                psb = ps_bc.tile([128, E], F32, tag="psb")
                halves = ((0, 256), (256, E)) if g == 0 else ((0, E),)
                for c0, c1 in halves:
                    nc.tensor.matmul(psb[:, c0:c1],
                                     pkw_t[:, 64 + 128 * g:64 + 128 * (g + 1)],
                                     h1_t[:, c0:c1], start=c0 == 0,
                                     stop=c1 == E, skip_group_check=True)
                    if g == 0 and c0 == 0:
                        # DVE computes the first quarter's relu itself so its
                        # stream starts without waiting on the ACT queue
                        nc.vector.tensor_scalar(
                            bct[:, 0:256], psb[:, 0:256],
                            sv_t[:, 4 + g:5 + g], 0.0,
                            mybir.AluOpType.add, mybir.AluOpType.max)
                    else:
                        nc.scalar.activation(bct[:, c0:c1], psb[:, c0:c1],
                                             mybir.ActivationFunctionType.Relu,
                                             bias=sv_t[:, 4 + g:5 + g],
                                             scale=1.0)
            else:
                src = h2_t[2 * g:2 * g + 2, :]
                src_b = dataclasses.replace(src, ap=[src.ap[0], [0, 64], [1, E]])
                nc.scalar.dma_start(bct[:], src_b)

            pt = pt_pool.tile([128, 4 * E], BF16, tag="pt")
            # chunk 0's DVE multiply is split so the AB matmuls (blocks 0/1)
            # can start while blocks 2/3 are still being produced. Pool runs
            # ~200ns/chunk ahead of DVE, so for the last 4 chunks blocks 2-3
            # shift to Pool and both engines finish the KR stream early.
            if g == 0:
                dve_parts, pool_parts = (), ()
                for c0, c1 in ((0, 256), (256, E)):
                    for b0_, b1_ in ((0, 2), (2, 4)):
                        nb = b1_ - b0_
                        bctb = dataclasses.replace(
                            bct[:, c0:c1],
                            ap=[bct[:, c0:c1].ap[0], [0, nb], [1, c1 - c0]])
                        nc.vector.tensor_mul(
                            pt[:, b0_ * E:b1_ * E].rearrange(
                                "p (b e) -> p b e", b=nb)[:, :, c0:c1],
                            fin0_t[:, b0_ * E:b1_ * E].rearrange(
                                "p (b e) -> p b e", b=nb)[:, :, c0:c1], bctb)
            elif g >= G - 5:
                dve_parts, pool_parts = ((0, 2),), ((2, 4),)
            else:
                dve_parts, pool_parts = ((0, 4),), ()
            for b0_, b1_ in dve_parts:
                nb = b1_ - b0_
                bctb = dataclasses.replace(
                    bct[:], ap=[bct[:].ap[0], [0, nb], [1, E]])
                nc.vector.tensor_mul(
                    pt[:, b0_ * E:b1_ * E].rearrange("p (b e) -> p b e", b=nb),
                    fin0_t[:, b0_ * E:b1_ * E].rearrange(
                        "p (b e) -> p b e", b=nb), bctb)
            ecuts = ((0, 256), (256, E)) if g == G - 1 else ((0, E),)
            for b0_, b1_ in pool_parts:
                nb = b1_ - b0_
                for c0, c1 in ecuts:
                    bctb = dataclasses.replace(
                        bct[:, c0:c1],
                        ap=[bct[:, c0:c1].ap[0], [0, nb], [1, c1 - c0]])
                    nc.gpsimd.tensor_mul(
                        pt[:, b0_ * E:b1_ * E].rearrange(
                            "p (b e) -> p b e", b=nb)[:, :, c0:c1],
                        fin0_t[:, b0_ * E:b1_ * E].rearrange(
                            "p (b e) -> p b e", b=nb)[:, :, c0:c1], bctb)
            if g % 2 == 0:
                pt45_cur[0] = pt5_pool.tile([128, 4 * E], F8, tag="pt45",
                                            name="pt45")
            ptf = pt45_cur[0]
            for c0, c1 in ecuts:
                dst = dataclasses.replace(
                    ptf[:, (g % 2) * E + c0:(g % 2) * E + c1],
                    ap=[ptf[:].ap[0], [2 * E, 2], [1, c1 - c0]])
                bct2 = dataclasses.replace(
                    bct[:, c0:c1],
                    ap=[bct[:, c0:c1].ap[0], [0, 2], [1, c1 - c0]])
                f45 = dataclasses.replace(
                    fin0_t[:, 4 * E + c0:4 * E + c1],
                    ap=[fin0_t[:].ap[0], [E, 2], [1, c1 - c0]])
                nc.gpsimd.tensor_mul(dst, f45, bct2)
            return pt, ptf[:, 0:2 * E], ptf[:, 2 * E:4 * E]

        # prefill so PE never waits on the broadcast/multiply chain mid-loop;
        # layer 2 (h2, feeding the DMA-replicated chunks) is slotted between
        # the first prefill chunks so it stays off the critical path.
        pre = [emit_pt(0), emit_pt(1)]
        ps_h2 = ps_mlp.tile([64, E], F32, name="ps_h2")
        nc.tensor.matmul(ps_h2[:], pkw_t[:, 0:64], h1_t[:], start=True, stop=True)
        nc.scalar.activation(h2_t[:], ps_h2[:], mybir.ActivationFunctionType.Relu,
                             bias=sv_t[0:64, 5 + KPE:6 + KPE], scale=1.0)
        pre += [emit_pt(g) for g in range(2, KPE)]

        # per sample block finisher: openers (b2 bias terms + output bias via
        # the ones row; accumulation is commutative so they close each group),
        # then the ACT/DVE epilogue:
        # out0 = A*s2 + (I3*D + bias) ; out1_i = B*v2_i + C_i
        mult_op = mybir.AluOpType.mult
        add_op = mybir.AluOpType.add

        tAs, tBs = [], []

        def finish_eb(eb):
            # phase 1: per-partition scale factors into bf16 staging tiles
            tA = work.tile([128, 128], BF16, tag=f"tA{eb}", name=f"tA{eb}")
            nc.scalar.mul(tA[:], acc[eb][:, 0:128], sv_t[:, eb:eb + 1])
            tB = work.tile([128, 192], BF16, tag=f"tB{eb}", name=f"tB{eb}")
            accB = acc[eb][:, 128:192]
            accB3 = dataclasses.replace(accB, ap=[accB.ap[0], [0, 3], [1, 64]])
            nc.vector.tensor_mul(
                tB[:].rearrange("p (i w) -> p i w", i=3),
                v2b3_t[:, eb * 192:(eb + 1) * 192].rearrange(
                    "p (i w) -> p i w", i=3), accB3)
            tAs.append(tA); tBs.append(tB)

        def finish2_eb(eb):
            # phase 2: identity matmuls add the scaled terms into the PSUM
            # accumulators (PE is cheap: cost = output columns), then plain
            # ACT/DVE evacuations feed the output DMA
            sl = bass.ts(eb, 128)
            ident = pk128_t[:, E + 192:E + 320]
            nc.tensor.matmul(acc[eb][:, 384:512], ident, tAs[eb][:],
                             start=False, stop=False, skip_group_check=True)
            nc.tensor.matmul(acc[eb][:, 192:384], ident, tBs[eb][:],
                             start=False, stop=False, skip_group_check=True)
            outS = out_pool.tile([128, 320], F32, tag="outS")
            nc.scalar.copy(outS[:, 0:128], acc[eb][:, 384:512])
            o1v = outS[:, 128:320].rearrange("p (w i) -> p i w", i=3)
            nc.vector.tensor_copy(
                o1v, acc[eb][:, 192:384].rearrange("p (i w) -> p i w", i=3))
            (nc.sync if eb % 2 == 0 else nc.scalar).dma_start(
                outp_d[sl, :], outS[:])

        def emit_openers():
            # b2 bias terms + output bias (ones row): accumulation is
            # commutative, so these run mid-loop once their params are loaded
            for eb in range(EB):
                sl = bass.ts(eb, 128)
                nc.tensor.matmul(acc[eb][:, 0:192], pk128_t[:, sl],
                                 pk128_t[:, E:E + 192], start=False,
                                 stop=False, skip_group_check=True)
                for i in range(3):
                    nc.tensor.matmul(acc[eb][:, 192 + 64 * i:256 + 64 * i],
                                     fin0_t[0:64, (2 + i) * E + eb * 128:
                                            (2 + i) * E + eb * 128 + 128],
                                     pk65_t[0:64, E + 128:E + 192],
                                     start=False, stop=False,
                                     skip_group_check=True)
                nc.tensor.matmul(acc[eb][:, 384:512], pk65_t[:, sl],
                                 pk65_t[:, E:E + 128], start=False,
                                 stop=False, skip_group_check=True)

        # main loop; the last chunk interleaves each block's finisher so the
        # epilogue of earlier blocks overlaps the remaining PE work
        for g in range(G):
            pt, pt4, pt5 = pre[g] if g < KPE else emit_pt(g)
            first = g == 0
            last = g == G - 1
            for eb in range(EB):
                ebo = eb * 128

                def pts(j):
                    return pt[:, j * E + ebo:j * E + ebo + 128]

                nc.tensor.matmul(acc[eb][:, 0:192], pts(1),
                                 wab_hi_t[:, bass.ts(g, 192)], start=first,
                                 stop=False, skip_group_check=True)
                nc.tensor.matmul(acc[eb][:, 0:192], pts(0),
                                 wab_lo_t[:, bass.ts(g, 192)], start=False,
                                 stop=last, skip_group_check=True)
                # NOTE: start=True marks the whole 2KB PSUM bank pending-zero,
                # so only the bank's very first matmul (ABhi at g=0) sets it;
                # other regions' first writes then overwrite-on-pending.
                for i in range(2):
                    nc.tensor.matmul(acc[eb][:, 192 + 64 * i:256 + 64 * i],
                                     pts(2 + i), wcc_t[:, bass.ts(g, 64)],
                                     start=False, stop=last,
                                     skip_group_check=True)
                if g % 2 == 1:
                    # fp8 DoubleRow: one matmul covers the chunk pair for
                    # both the C2 and D paths
                    gp = g // 2
                    nc.tensor.matmul(
                        acc[eb][:, 320:384],
                        pt4.rearrange(
                            "p (c e) -> p c e", c=2)[:, :, ebo:ebo + 128],
                        wc2_t[:, 128 * gp:128 * (gp + 1)].rearrange(
                            "p (c w) -> p c w", c=2),
                        start=False, stop=last, skip_group_check=True,
                        perf_mode=mybir.MatmulPerfMode.DoubleRow)
                    nc.tensor.matmul(
                        acc[eb][:, 384:512],
                        pt5.rearrange(
                            "p (c e) -> p c e", c=2)[:, :, ebo:ebo + 128],
                        wdd_t[:, 256 * gp:256 * (gp + 1)].rearrange(
                            "p (c w) -> p c w", c=2),
                        start=False, stop=last, skip_group_check=True,
                        perf_mode=mybir.MatmulPerfMode.DoubleRow)
                if last:
                    finish_eb(eb)
            if g == 10:
                emit_openers()
        for eb in range(EB):
            finish2_eb(eb)

    nc.compile()
    return nc


_NC = None


def _get_nc():
    global _NC
    if _NC is None:
        _NC = _build_nc()
    return _NC


def _prep_inputs(data_in1, data_in2, weight, W0, b0, W1, b1, W2, b2, bias):
    f32 = np.float32
    data_in1 = np.ascontiguousarray(data_in1, dtype=f32)
    data_in2 = np.ascontiguousarray(data_in2, dtype=f32)
    weight = np.ascontiguousarray(weight, dtype=f32)
    W0 = np.asarray(W0, f32); b0 = np.asarray(b0, f32)
    W1 = np.asarray(W1, f32); b1 = np.asarray(b1, f32)
    W2 = np.asarray(W2, f32); b2 = np.asarray(b2, f32)
    bias = np.asarray(bias, f32)

    s1 = data_in1[:, :MUL0]                      # [N,128]
    v1 = data_in1[:, MUL0:].reshape(N, MUL1, 3)  # [N,64,3]
    s2 = data_in2[:, 0]                          # [N]
    v2 = data_in2[:, 1:4]                        # [N,3]

    def bf(x):
        return np.ascontiguousarray(x, dtype=f32).astype(BF16_NP)

    s1t = s1.T                                   # [128,N] f32
    # fused KR x-operand blocks, each [128, N]
    s1lo = np.concatenate([s1t[0:64], s1t[0:64]], axis=0)
    s1hi = np.concatenate([s1t[64:128], s1t[64:128]], axis=0)
    vs = []
    for i in range(3):
        v1s2 = (v1[:, :, i] * s2[:, None]).T     # [64,N]
        vs.append(np.concatenate([v1s2, v1s2], axis=0))
    dot12 = np.einsum("eui,ei->eu", v1, v2).T    # [64,N]
    d2 = np.concatenate([dot12, dot12], axis=0)
    fin0 = bf(np.stack([s1lo, s1hi, vs[0], vs[1], vs[2], d2], axis=1))
    # fin0: [128, 6, N]
    d2o = bf(np.concatenate([dot12, np.ones((1, N), f32)], axis=0))  # [65,N]
    wT = bf(weight.T)

    # W2 chunk layouts: chunk g rows r=(koff*64+uu) -> W2x[2g+koff, sel(uu), :]
    def chunks(arr3, usel):  # arr3 [64,U,W] -> [128, G, W]
        a = arr3.reshape(G, 2, arr3.shape[1], arr3.shape[2])[:, :, usel, :]
        return np.transpose(a, (1, 2, 0, 3)).reshape(128, G, arr3.shape[2])

    Wa3 = W2[:, :N1].reshape(64, 128, 128)
    Wb3 = W2[:, N1:N1 + N2].reshape(64, 128, 64)
    Wc3 = W2[:, N1 + N2:N1 + N2 + N3].reshape(64, 64, 64)
    Wd3 = W2[:, N1 + N2 + N3:].reshape(64, 64, 128)
    lo, hi = slice(0, 64), slice(64, 128)

    def ab(usel):  # [128, G*192]: per chunk [Wa(128) | Wb(64)]
        return bf(np.concatenate(
            [chunks(Wa3, usel), chunks(Wb3, usel)], axis=2
        ).reshape(128, G * 192))

    # wg1K[m, 128g+r] = W1[m, 2g + r//64]: replicated W1 columns so the PE
    # broadcast for the first KPE chunks fuses layer 2 (works off h1)
    wg1K = np.repeat(W1, 64, axis=1)[:, :128 * KPE]
    # bg1K[p, g] = b1[2g + p//64]
    bg1K = np.concatenate(
        [np.broadcast_to(b1[0:2 * KPE:2], (64, KPE)),
         np.broadcast_to(b1[1:2 * KPE:2], (64, KPE))], axis=0).astype(f32)

    shared = {
        "pkw": bf(np.concatenate([W1, wg1K], axis=1)),
        "wab_lo": ab(lo),
        "wab_hi": ab(hi),
        "wcc": bf(chunks(Wc3, lo).reshape(128, G * 64)),
        "wc2": np.ascontiguousarray(
            chunks(Wc3, lo).reshape(128, G * 64), f32).astype(F8_NP),
        "wdd": np.ascontiguousarray(
            I3 * chunks(Wd3, lo).reshape(128, G * 128), f32).astype(F8_NP),
    }
    bab = np.concatenate(
        [b2[:N1].reshape(128, 128), b2[N1:N1 + N2].reshape(128, 64)], axis=1)
    bdb = np.concatenate(
        [I3 * b2[N1 + N2 + N3:].reshape(64, 128), bias[None, :]], axis=0)
    bc2p = np.concatenate(
        [b2[N1 + N2:N1 + N2 + N3].reshape(64, 64), np.zeros((1, 64), f32)],
        axis=0)

    in_maps = []
    for c in range(N_CORES):
        e0 = c * E
        m = dict(shared)
        m["fin0"] = np.ascontiguousarray(
            fin0[:, :, e0:e0 + E]).reshape(128, 6 * E)
        m["pk16"] = bf(np.concatenate([W0, wT[:, e0:e0 + E]], axis=1))
        m["pk65"] = bf(np.concatenate([d2o[:, e0:e0 + E], bdb, bc2p], axis=1))
        m["pk128"] = bf(np.concatenate(
            [s1t[:, e0:e0 + E], bab, np.eye(128, dtype=f32)], axis=1))
        # sv: cols 0:4 = s2 per sample block; then bg1K, then b0, b1
        sv = np.zeros((128, 6 + KPE), f32)
        sv[:, 4:4 + KPE] = bg1K
        sv[0:64, 4 + KPE] = b0
        sv[0:64, 5 + KPE] = b1
        v2b3 = np.zeros((128, EB, 3, 64), f32)
        for eb in range(EB):
            b0_ = e0 + eb * 128
            sv[:, eb] = s2[b0_:b0_ + 128]
            for i in range(3):
                v2b3[:, eb, i, :] = v2[b0_:b0_ + 128, i:i + 1]
        m["sv"] = sv
        m["v2b3"] = v2b3.reshape(128, EB * 192)
        in_maps.append(m)
    return in_maps


def run(in_maps, **kwargs):
    nc = _get_nc()
    return run_bass_kernel_spmd(nc, in_maps, list(range(N_CORES)), **kwargs)


def kernel(data_in1, data_in2, weight, W0, b0, W1, b1, W2, b2, bias):
    in_maps = _prep_inputs(
        data_in1, data_in2, weight, W0, b0, W1, b1, W2, b2, bias
    )
    res = run(in_maps)
    out = np.concatenate(
        [np.asarray(res.results[c]["outp"]) for c in range(N_CORES)], axis=0
    )
    return out.astype(np.float32)
